# revision 2
# baseline (speedup 1.0000x reference)
"""Trainium2 Bass kernel for nn_EpisA (sparse_attention, 8 NeuronCores).

Math (reference):
    signal = relu(x)                                   (n,h,l)
    Ss     = 1 - cumsum(signal, -1)                    (n,h,l)
    Is[i]  = sum_{j<=i} r^(i-j) * signal[j],  r = 1 - 1/tau   -> linear scan
    alpha  = 1 - exp(-R0dTaus * Is)
    Alpha[k] = sum_m (Amat[k,m] + I[k,m]) * alpha[m]   (node mixing matmul)
    predSignal = Alpha * Ss
    outputs: (predSignal, signal, Amat + I)

Device strategy (identical SPMD program on 8 cores, no collectives — the
problem is tiny (~3 MB I/O) so a collective's latency floor would dominate):
  * every core computes the row-wise pipeline (relu / Is-scan / exp / Ss-scan)
    for all 256 (n,h) rows: scans run on the Vector engine via
    tensor_tensor_scan, transcendentals on the Scalar engine, relu on GPSIMD.
  * the node-mixing matmul is sharded over L: each core contracts only its own
    128-column chunk of e = exp(-R0d*Is), selected with a partition_id()-based
    dynamic slice.  Alpha = stat @ 1 - stat @ e via two accumulating matmuls.
  * signal / tempAmat.T outputs are sharded over node rows: inputs are rolled
    per core so each core's shard sits at partitions [0:16] of its tiles.
Host side only reorders/shards inputs and reassembles the three outputs.
"""

import numpy as np

for _p in ("/opt/trn_rl_repo", "/root/.axon_site/_ro/trn_rl_repo"):
    try:
        import concourse  # noqa: F401
        break
    except ImportError:
        import sys
        if _p not in sys.path:
            sys.path.insert(0, _p)

N, H, L = 128, 2, 1024
NC = 8            # cores
S = N // NC       # 16 node rows per core (signal / tempAmat.T shards)
LC = L // NC      # 128 L columns per core (predSignal shard)

_CACHE = {}


def _build():
    """Build + compile the (single, SPMD-identical) Bass program."""
    from concourse import bacc, bass, mybir, tile

    f32 = mybir.dt.float32
    Alu = mybir.AluOpType
    Act = mybir.ActivationFunctionType

    nc = bacc.Bacc(
        "TRN2", target_bir_lowering=False, debug=False, num_devices=NC
    )

    xh = nc.dram_tensor("xh", [H, N, L], f32, kind="ExternalInput").ap()
    tausr = nc.dram_tensor("tausr", [N, H], f32, kind="ExternalInput").ap()
    r0dr = nc.dram_tensor("r0dr", [N, H], f32, kind="ExternalInput").ap()
    statm = nc.dram_tensor("statm", [N, N], f32, kind="ExternalInput").ap()
    eyem = nc.dram_tensor("eyem", [N, N], f32, kind="ExternalInput").ap()
    arow = nc.dram_tensor("arow", [S, N], f32, kind="ExternalInput").ap()
    erow = nc.dram_tensor("erow", [S, N], f32, kind="ExternalInput").ap()

    pred_o = nc.dram_tensor("pred_o", [H, N, LC], f32, kind="ExternalOutput").ap()
    sig_o = nc.dram_tensor("sig_o", [H, S, L], f32, kind="ExternalOutput").ap()
    amt_o = nc.dram_tensor("amt_o", [S, N], f32, kind="ExternalOutput").ap()

    with tile.TileContext(nc) as tc:
        with (
            tc.tile_pool(name="p", bufs=1) as pool,
            tc.tile_pool(name="ps", bufs=2, space="PSUM") as psp,
        ):
            def t(name, shape):
                return pool.tile(shape, f32, name=name, tag=name)

            taus_t = t("taus", [N, H]); nc.sync.dma_start(taus_t, tausr)
            r0d_t = t("r0d", [N, H]); nc.sync.dma_start(r0d_t, r0dr)
            statm_t = t("statm", [N, N]); nc.sync.dma_start(statm_t, statm)
            eyem_t = t("eyem", [N, N]); nc.sync.dma_start(eyem_t, eyem)
            arow_t = t("arow", [S, N]); nc.sync.dma_start(arow_t, arow)
            erow_t = t("erow", [S, N]); nc.sync.dma_start(erow_t, erow)

            # r = 1 - 1/tau per (rolled) row, column h; -R0dTaus likewise.
            inv_t = t("inv", [N, H])
            nc.vector.reciprocal(inv_t, taus_t)
            rcol = t("rcol", [N, H])
            nc.vector.tensor_scalar(rcol, inv_t, -1.0, 1.0, Alu.mult, Alu.add)
            nr0d = t("nr0d", [N, H])
            nc.vector.tensor_scalar(nr0d, r0d_t, -1.0, None, Alu.mult)

            # stationary stat[m,k] = Amat[k,m] + I (doubly-rolled layout) and
            # its negation for the  Alpha = stat@1 - stat@e  PSUM accumulation.
            stat = t("stat", [N, N])
            nc.vector.tensor_tensor(stat, statm_t, eyem_t, Alu.add)
            statn = t("statn", [N, N])
            nc.vector.scalar_tensor_tensor(
                statn, statm_t, -1.0, eyem_t, Alu.mult, Alu.subtract
            )

            # tempAmat.T output rows = Amat[shard,:] + I[shard,:]
            amrow = t("amrow", [S, N])
            nc.vector.tensor_tensor(amrow, arow_t, erow_t, Alu.add)
            nc.sync.dma_start(amt_o, amrow)

            ones = t("ones", [N, L])
            nc.gpsimd.memset(ones, 1.0)

            # dynamic L-chunk offset = 128 * core_id (Vector engine reads it)
            off = nc.vector.partition_id() * LC

            for h in range(H):
                x_t = t(f"x{h}", [N, L])
                nc.sync.dma_start(x_t, xh[h])

                sig_t = t(f"sig{h}", [N, L])
                nc.gpsimd.tensor_scalar(sig_t, x_t, 0.0, None, Alu.max)  # relu
                nc.sync.dma_start(sig_o[h], sig_t[0:S, :])

                # r broadcast along L, then Is[i] = r*Is[i-1] + sig[i]
                rt_t = t(f"rt{h}", [N, L])
                nc.scalar.mul(rt_t, ones, rcol[:, h:h + 1])
                is_t = t(f"is{h}", [N, L])
                nc.vector.tensor_tensor_scan(
                    is_t, rt_t, sig_t, 0.0, Alu.mult, Alu.add
                )

                e_t = t(f"e{h}", [N, L])
                nc.scalar.activation(e_t, is_t, Act.Exp, scale=nr0d[:, h:h + 1])

                # Ss[i] = Ss[i-1] - sig[i], Ss[-1] = 1   (= 1 - cumsum)
                ss_t = t(f"ss{h}", [N, L])
                nc.vector.tensor_tensor_scan(
                    ss_t, ones, sig_t, 1.0, Alu.mult, Alu.subtract
                )

                # own L-chunk of e, then Alpha = stat@1 - stat@e into PSUM
                ech_t = t(f"ech{h}", [N, LC])
                nc.vector.tensor_copy(ech_t, e_t[:, bass.ds(off, LC)])
                ps = psp.tile([N, LC], f32, name=f"ps{h}", tag=f"ps{h}")
                nc.tensor.matmul(ps, lhsT=stat, rhs=ones[:, 0:LC],
                                 start=True, stop=False)
                nc.tensor.matmul(ps, lhsT=statn, rhs=ech_t,
                                 start=False, stop=True)

                pred_t = t(f"pred{h}", [N, LC])
                nc.vector.tensor_tensor(
                    pred_t, ps, ss_t[:, bass.ds(off, LC)], Alu.mult
                )
                nc.sync.dma_start(pred_o[h], pred_t)

    nc.compile()
    return nc


def _in_maps(x, Amat, taus, R0dTaus):
    """Per-core host-side sharding: pure slicing/rolling, no math."""
    x = np.ascontiguousarray(x, dtype=np.float32)
    Amat = np.ascontiguousarray(Amat, dtype=np.float32)
    taus = np.ascontiguousarray(taus, dtype=np.float32)
    R0dTaus = np.ascontiguousarray(R0dTaus, dtype=np.float32)
    xt = x.transpose(1, 0, 2)          # (H, N, L)
    At = Amat.T.copy()                 # At[m, k] = Amat[k, m]
    eye = np.eye(N, dtype=np.float32)
    maps = []
    for c in range(NC):
        r = S * c
        maps.append({
            # row-rolled so this core's node shard sits at partitions [0:S)
            "xh": np.ascontiguousarray(np.roll(xt, -r, axis=1)),
            "tausr": np.ascontiguousarray(np.roll(taus, -r, axis=0)),
            "r0dr": np.ascontiguousarray(np.roll(R0dTaus, -r, axis=0)),
            # doubly-rolled Amat.T so PSUM row kk maps to global k=(kk+r)%N,
            # matching the rolled Ss rows; eye is roll-invariant.
            "statm": np.ascontiguousarray(
                np.roll(np.roll(At, -r, axis=0), -r, axis=1)),
            "eyem": eye,
            "arow": np.ascontiguousarray(Amat[r:r + S, :]),
            "erow": np.ascontiguousarray(eye[r:r + S, :]),
        })
    return maps


def _assemble(results):
    pred = np.empty((N, H, L), dtype=np.float32)
    signal = np.empty((N, H, L), dtype=np.float32)
    amatT = np.empty((N, N), dtype=np.float32)
    for c, res in enumerate(results):
        r = S * c
        for h in range(H):
            # pred_o rows are in rolled-k order; roll back by +r
            pred[:, h, LC * c:LC * (c + 1)] = np.roll(res["pred_o"][h], r, axis=0)
            signal[r:r + S, h, :] = res["sig_o"][h]
        amatT[r:r + S, :] = res["amt_o"]
    return pred, signal, amatT


def kernel(x, Amat, taus, R0dTaus):
    from concourse import bass_utils

    if "nc" not in _CACHE:
        _CACHE["nc"] = _build()
    res = bass_utils.run_bass_kernel_spmd(
        _CACHE["nc"], _in_maps(x, Amat, taus, R0dTaus), core_ids=list(range(NC))
    )
    return _assemble(res.results)


# revision 4
# speedup vs baseline: 1.9443x; 1.9443x over previous
"""Trainium2 Bass kernel for nn_EpisA (sparse_attention, 8 NeuronCores).

Math (reference):
    signal = relu(x)                                   (n,h,l)
    Ss     = 1 - cumsum(signal, -1)                    (n,h,l)
    Is[i]  = sum_{j<=i} r^(i-j) * signal[j],  r = 1 - 1/tau   -> linear scan
    alpha  = 1 - exp(-R0dTaus * Is)
    Alpha[k] = sum_m (Amat[k,m] + I[k,m]) * alpha[m]   (node mixing matmul)
    predSignal = Alpha * Ss
    outputs: (predSignal, signal, Amat + I)

Device strategy (identical SPMD program on 8 cores, no collectives — the
problem is tiny (~3 MB I/O) so a collective's latency floor would dominate):
  * every core computes the row-wise pipeline (relu / Is-scan / exp / Ss-scan)
    for all 256 (n,h) rows: scans run on the Vector engine via
    tensor_tensor_scan, transcendentals on the Scalar engine, relu on GPSIMD.
  * the node-mixing matmul is sharded over L: each core contracts only its own
    128-column chunk of e = exp(-R0d*Is), selected with a partition_id()-based
    dynamic slice.  Alpha = stat @ 1 - stat @ e via two accumulating matmuls.
  * signal / tempAmat.T outputs are sharded over node rows: inputs are rolled
    per core so each core's shard sits at partitions [0:16] of its tiles.
Host side only reorders/shards inputs and reassembles the three outputs.
"""

import numpy as np

for _p in ("/opt/trn_rl_repo", "/root/.axon_site/_ro/trn_rl_repo"):
    try:
        import concourse  # noqa: F401
        break
    except ImportError:
        import sys
        if _p not in sys.path:
            sys.path.insert(0, _p)

N, H, L = 128, 2, 1024
NC = 8            # cores
S = N // NC       # 16 node rows per core (signal / tempAmat.T shards)
LC = L // NC      # 128 L columns per core (predSignal shard)

_CACHE = {}


def _build():
    """Build + compile the (single, SPMD-identical) Bass program."""
    from concourse import bacc, bass, mybir, tile

    f32 = mybir.dt.float32
    Alu = mybir.AluOpType
    Act = mybir.ActivationFunctionType

    nc = bacc.Bacc(
        "TRN2", target_bir_lowering=False, debug=False, num_devices=NC
    )

    xh = nc.dram_tensor("xh", [H, N, L], f32, kind="ExternalInput").ap()
    tausr = nc.dram_tensor("tausr", [N, H], f32, kind="ExternalInput").ap()
    r0dr = nc.dram_tensor("r0dr", [N, H], f32, kind="ExternalInput").ap()
    statm = nc.dram_tensor("statm", [N, N], f32, kind="ExternalInput").ap()
    eyem = nc.dram_tensor("eyem", [N, N], f32, kind="ExternalInput").ap()
    arow = nc.dram_tensor("arow", [S, N], f32, kind="ExternalInput").ap()
    erow = nc.dram_tensor("erow", [S, N], f32, kind="ExternalInput").ap()

    pred_o = nc.dram_tensor("pred_o", [H, N, LC], f32, kind="ExternalOutput").ap()
    sig_o = nc.dram_tensor("sig_o", [H, S, L], f32, kind="ExternalOutput").ap()
    amt_o = nc.dram_tensor("amt_o", [S, N], f32, kind="ExternalOutput").ap()

    import bass_rust

    def bcast(col_ap, n):
        """(P,1) column AP -> (P,n) stride-0 broadcast AP along free dim."""
        newap = [list(p) for p in col_ap.ap]
        newap[-1] = [0, n]
        return bass_rust.AP(col_ap.tensor, col_ap.offset, newap)

    with tile.TileContext(nc) as tc:
        with (
            tc.tile_pool(name="p", bufs=1) as pool,
            tc.tile_pool(name="ps", bufs=2, space="PSUM") as psp,
        ):
            def t(name, shape):
                return pool.tile(shape, f32, name=name, tag=name)

            taus_t = t("taus", [N, H]); nc.sync.dma_start(taus_t, tausr)
            r0d_t = t("r0d", [N, H]); nc.sync.dma_start(r0d_t, r0dr)
            statm_t = t("statm", [N, N]); nc.sync.dma_start(statm_t, statm)
            eyem_t = t("eyem", [N, N]); nc.sync.dma_start(eyem_t, eyem)
            arow_t = t("arow", [S, N]); nc.sync.dma_start(arow_t, arow)
            erow_t = t("erow", [S, N]); nc.sync.dma_start(erow_t, erow)

            # r = 1 - 1/tau per (rolled) row, column h; -R0dTaus likewise.
            inv_t = t("inv", [N, H])
            nc.vector.reciprocal(inv_t, taus_t)
            rcol = t("rcol", [N, H])
            nc.vector.tensor_scalar(rcol, inv_t, -1.0, 1.0, Alu.mult, Alu.add)
            nr0d = t("nr0d", [N, H])
            nc.vector.tensor_scalar(nr0d, r0d_t, -1.0, None, Alu.mult)

            # stationary stat[m,k] = Amat[k,m] + I (doubly-rolled layout) and
            # its negation for the  Alpha = stat@1 - stat@e  PSUM accumulation.
            stat = t("stat", [N, N])
            nc.vector.tensor_tensor(stat, statm_t, eyem_t, Alu.add)
            statn = t("statn", [N, N])
            nc.vector.scalar_tensor_tensor(
                statn, statm_t, -1.0, eyem_t, Alu.mult, Alu.subtract
            )

            # tempAmat.T output rows = Amat[shard,:] + I[shard,:]
            amrow = t("amrow", [S, N])
            nc.vector.tensor_tensor(amrow, arow_t, erow_t, Alu.add)
            nc.sync.dma_start(amt_o, amrow)

            onec = t("onec", [N, 1])
            nc.gpsimd.memset(onec, 1.0)          # scan data0 broadcast source
            ones = t("ones", [N, LC])
            nc.gpsimd.memset(ones, 1.0)          # matmul "@1" moving operand

            # dynamic L-chunk offset = 128 * core_id (registers on all engines)
            off = nc.partition_id() * LC

            for h in range(H):
                x_t = t(f"x{h}", [N, L])
                nc.sync.dma_start(x_t, xh[h])

                sig_t = t(f"sig{h}", [N, L])
                nc.scalar.activation(sig_t, x_t, Act.Relu)
                nc.sync.dma_start(sig_o[h], sig_t[0:S, :])

                # Is[i] = r*Is[i-1] + sig[i]   (r broadcast via stride-0 AP)
                is_t = t(f"is{h}", [N, L])
                nc.vector.tensor_tensor_scan(
                    is_t, bcast(rcol[:, h:h + 1], L), sig_t, 0.0,
                    Alu.mult, Alu.add
                )

                e_t = t(f"e{h}", [N, L])
                nc.scalar.activation(e_t, is_t, Act.Exp, scale=nr0d[:, h:h + 1])

                # Ss[i] = Ss[i-1] - sig[i], Ss[-1] = 1   (= 1 - cumsum)
                ss_t = t(f"ss{h}", [N, L])
                nc.vector.tensor_tensor_scan(
                    ss_t, bcast(onec[:, 0:1], L), sig_t, 1.0,
                    Alu.mult, Alu.subtract
                )

                # Alpha = stat@1 - stat@e  accumulated in PSUM (own L-chunk)
                ps = psp.tile([N, LC], f32, name=f"ps{h}", tag=f"ps{h}")
                nc.tensor.matmul(ps, lhsT=stat, rhs=ones,
                                 start=True, stop=False)
                nc.tensor.matmul(ps, lhsT=statn, rhs=e_t[:, bass.ds(off, LC)],
                                 start=False, stop=True)

                pred_t = t(f"pred{h}", [N, LC])
                nc.vector.tensor_tensor(
                    pred_t, ps, ss_t[:, bass.ds(off, LC)], Alu.mult
                )
                nc.sync.dma_start(pred_o[h], pred_t)

    nc.compile()
    return nc


def _in_maps(x, Amat, taus, R0dTaus):
    """Per-core host-side sharding: pure slicing/rolling, no math."""
    x = np.ascontiguousarray(x, dtype=np.float32)
    Amat = np.ascontiguousarray(Amat, dtype=np.float32)
    taus = np.ascontiguousarray(taus, dtype=np.float32)
    R0dTaus = np.ascontiguousarray(R0dTaus, dtype=np.float32)
    xt = x.transpose(1, 0, 2)          # (H, N, L)
    At = Amat.T.copy()                 # At[m, k] = Amat[k, m]
    eye = np.eye(N, dtype=np.float32)
    maps = []
    for c in range(NC):
        r = S * c
        maps.append({
            # row-rolled so this core's node shard sits at partitions [0:S)
            "xh": np.ascontiguousarray(np.roll(xt, -r, axis=1)),
            "tausr": np.ascontiguousarray(np.roll(taus, -r, axis=0)),
            "r0dr": np.ascontiguousarray(np.roll(R0dTaus, -r, axis=0)),
            # doubly-rolled Amat.T so PSUM row kk maps to global k=(kk+r)%N,
            # matching the rolled Ss rows; eye is roll-invariant.
            "statm": np.ascontiguousarray(
                np.roll(np.roll(At, -r, axis=0), -r, axis=1)),
            "eyem": eye,
            "arow": np.ascontiguousarray(Amat[r:r + S, :]),
            "erow": np.ascontiguousarray(eye[r:r + S, :]),
        })
    return maps


def _assemble(results):
    pred = np.empty((N, H, L), dtype=np.float32)
    signal = np.empty((N, H, L), dtype=np.float32)
    amatT = np.empty((N, N), dtype=np.float32)
    for c, res in enumerate(results):
        r = S * c
        for h in range(H):
            # pred_o rows are in rolled-k order; roll back by +r
            pred[:, h, LC * c:LC * (c + 1)] = np.roll(res["pred_o"][h], r, axis=0)
            signal[r:r + S, h, :] = res["sig_o"][h]
        amatT[r:r + S, :] = res["amt_o"]
    return pred, signal, amatT


def kernel(x, Amat, taus, R0dTaus):
    from concourse import bass_utils

    if "nc" not in _CACHE:
        _CACHE["nc"] = _build()
    res = bass_utils.run_bass_kernel_spmd(
        _CACHE["nc"], _in_maps(x, Amat, taus, R0dTaus), core_ids=list(range(NC))
    )
    return _assemble(res.results)


# revision 10
# speedup vs baseline: 2.1007x; 1.0805x over previous
"""Trainium2 Bass kernel for nn_EpisA (sparse_attention, 8 NeuronCores).

Math (reference):
    signal = relu(x)                                   (n,h,l)
    Ss     = 1 - cumsum(signal, -1)                    (n,h,l)
    Is[i]  = sum_{j<=i} r^(i-j) * signal[j],  r = 1 - 1/tau   -> linear scan
    alpha  = 1 - exp(-R0dTaus * Is)
    Alpha[k] = sum_m (Amat[k,m] + I[k,m]) * alpha[m]   (node mixing matmul)
    predSignal = Alpha * Ss
    outputs: (predSignal, signal, Amat + I)

Device strategy (identical SPMD program on 8 cores, no collectives — the
problem is tiny (~3 MB I/O) so a collective's latency floor would dominate):
  * every core computes the row-wise pipeline (relu / Is-scan / exp / Ss-scan)
    for all 256 (n,h) rows: scans run on the Vector engine via
    tensor_tensor_scan, transcendentals on the Scalar engine, relu on GPSIMD.
  * the node-mixing matmul is sharded over L: each core contracts only its own
    128-column chunk of e = exp(-R0d*Is), selected with a partition_id()-based
    dynamic slice.  Alpha = stat @ 1 - stat @ e via two accumulating matmuls.
  * signal / tempAmat.T outputs are sharded over node rows: inputs are rolled
    per core so each core's shard sits at partitions [0:16] of its tiles.
Host side only reorders/shards inputs and reassembles the three outputs.
"""

import numpy as np

for _p in ("/opt/trn_rl_repo", "/root/.axon_site/_ro/trn_rl_repo"):
    try:
        import concourse  # noqa: F401
        break
    except ImportError:
        import sys
        if _p not in sys.path:
            sys.path.insert(0, _p)

N, H, L = 128, 2, 1024
NC = 8            # cores
S = N // NC       # 16 node rows per core (signal / tempAmat.T shards)
LC = L // NC      # 128 L columns per core (predSignal shard)

_CACHE = {}


def _build():
    """Build + compile the (single, SPMD-identical) Bass program."""
    from concourse import bacc, bass, mybir, tile

    f32 = mybir.dt.float32
    Alu = mybir.AluOpType
    Act = mybir.ActivationFunctionType

    nc = bacc.Bacc(
        "TRN2", target_bir_lowering=False, debug=False, num_devices=NC
    )

    xh = nc.dram_tensor("xh", [H, N, L], f32, kind="ExternalInput").ap()
    # merged scalars: [taus | R0dTaus] and [rolled Amat.T | eye], [Amat rows | I rows]
    scal4 = nc.dram_tensor("scal4", [N, 2 * H], f32, kind="ExternalInput").ap()
    statm2 = nc.dram_tensor("statm2", [N, 2 * N], f32, kind="ExternalInput").ap()
    arow2 = nc.dram_tensor("arow2", [S, 2 * N], f32, kind="ExternalInput").ap()

    pred_o = nc.dram_tensor("pred_o", [H, N, LC], f32, kind="ExternalOutput").ap()
    sig_o = nc.dram_tensor("sig_o", [H, S, L], f32, kind="ExternalOutput").ap()
    amt_o = nc.dram_tensor("amt_o", [S, N], f32, kind="ExternalOutput").ap()

    import bass_rust

    def bcast(col_ap, n):
        """(P,1) column AP -> (P,n) stride-0 broadcast AP along free dim."""
        newap = [list(p) for p in col_ap.ap]
        newap[-1] = [0, n]
        return bass_rust.AP(col_ap.tensor, col_ap.offset, newap)

    with tile.TileContext(nc) as tc:
        with (
            tc.tile_pool(name="p", bufs=1) as pool,
            tc.tile_pool(name="ps", bufs=2, space="PSUM") as psp,
        ):
            def t(name, shape):
                return pool.tile(shape, f32, name=name, tag=name)

            # x first: it gates the whole pipeline (relu -> scans). One issue
            # per h on the Sync HW-DGE queue; const loads go on the Scalar
            # sequencer so their ~600ns issue costs don't delay x.
            x_ts = []
            for h in range(H):
                x_t = t(f"x{h}", [N, L])
                nc.sync.dma_start(x_t, xh[h])
                x_ts.append(x_t)

            scal_t = t("scal", [N, 2 * H]); nc.scalar.dma_start(scal_t, scal4)
            statm2_t = t("statm2", [N, 2 * N]); nc.scalar.dma_start(statm2_t, statm2)
            arow2_t = t("arow2", [S, 2 * N]); nc.scalar.dma_start(arow2_t, arow2)
            taus_t = scal_t[:, 0:H]
            r0d_t = scal_t[:, H:2 * H]
            statm_t = statm2_t[:, 0:N]
            eyem_t = statm2_t[:, N:2 * N]
            arow_t = arow2_t[:, 0:N]
            erow_t = arow2_t[:, N:2 * N]

            # r = 1 - 1/tau per (rolled) row, column h; -R0dTaus likewise.
            inv_t = t("inv", [N, H])
            nc.vector.reciprocal(inv_t, taus_t)
            rcol = t("rcol", [N, H])
            nc.vector.tensor_scalar(rcol, inv_t, -1.0, 1.0, Alu.mult, Alu.add)
            nr0d = t("nr0d", [N, H])
            nc.vector.tensor_scalar(nr0d, r0d_t, -1.0, None, Alu.mult)

            # stationary stat[m,k] = Amat[k,m] + I (doubly-rolled layout) and
            # its negation for the  Alpha = stat@1 - stat@e  PSUM accumulation.
            stat = t("stat", [N, N])
            nc.vector.tensor_tensor(stat, statm_t, eyem_t, Alu.add)
            statn = t("statn", [N, N])
            nc.vector.scalar_tensor_tensor(
                statn, statm_t, -1.0, eyem_t, Alu.mult, Alu.subtract
            )

            # tempAmat.T output rows = Amat[shard,:] + I[shard,:]
            amrow = t("amrow", [S, N])
            nc.vector.tensor_tensor(amrow, arow_t, erow_t, Alu.add)
            nc.gpsimd.dma_start(amt_o, amrow)

            onec = t("onec", [N, 1])
            nc.gpsimd.memset(onec, 1.0)          # scan data0 broadcast source
            ones = t("ones", [N, LC])
            nc.gpsimd.memset(ones, 1.0)          # matmul "@1" moving operand

            # dynamic L-chunk offset = 128 * core_id (registers on all engines)
            off = nc.partition_id() * LC

            for h in range(H):
                x_t = x_ts[h]
                sig_t = t(f"sig{h}", [N, L])
                nc.scalar.activation(sig_t, x_t, Act.Relu)
                nc.gpsimd.dma_start(sig_o[h], sig_t[0:S, :])

                # Is[i] = r*Is[i-1] + sig[i]   (r broadcast via stride-0 AP)
                is_t = t(f"is{h}", [N, L])
                nc.vector.tensor_tensor_scan(
                    is_t, bcast(rcol[:, h:h + 1], L), sig_t, 0.0,
                    Alu.mult, Alu.add
                )

                e_t = t(f"e{h}", [N, L])
                nc.scalar.activation(e_t, is_t, Act.Exp, scale=nr0d[:, h:h + 1])

                # Ss[i] = Ss[i-1] - sig[i], Ss[-1] = 1   (= 1 - cumsum)
                ss_t = t(f"ss{h}", [N, L])
                nc.vector.tensor_tensor_scan(
                    ss_t, bcast(onec[:, 0:1], L), sig_t, 1.0,
                    Alu.mult, Alu.subtract
                )

                # Alpha = stat@1 - stat@e  accumulated in PSUM (own L-chunk)
                ps = psp.tile([N, LC], f32, name=f"ps{h}", tag=f"ps{h}")
                nc.tensor.matmul(ps, lhsT=stat, rhs=ones,
                                 start=True, stop=False)
                nc.tensor.matmul(ps, lhsT=statn, rhs=e_t[:, bass.ds(off, LC)],
                                 start=False, stop=True)

                pred_t = t(f"pred{h}", [N, LC])
                nc.vector.tensor_tensor(
                    pred_t, ps, ss_t[:, bass.ds(off, LC)], Alu.mult
                )
                nc.gpsimd.dma_start(pred_o[h], pred_t)

    nc.compile()
    return nc


def _in_maps(x, Amat, taus, R0dTaus):
    """Per-core host-side sharding: pure slicing/rolling, no math."""
    x = np.ascontiguousarray(x, dtype=np.float32)
    Amat = np.ascontiguousarray(Amat, dtype=np.float32)
    taus = np.ascontiguousarray(taus, dtype=np.float32)
    R0dTaus = np.ascontiguousarray(R0dTaus, dtype=np.float32)
    xt = x.transpose(1, 0, 2)          # (H, N, L)
    At = Amat.T.copy()                 # At[m, k] = Amat[k, m]
    eye = np.eye(N, dtype=np.float32)
    maps = []
    for c in range(NC):
        r = S * c
        maps.append({
            # row-rolled so this core's node shard sits at partitions [0:S)
            "xh": np.ascontiguousarray(np.roll(xt, -r, axis=1)),
            "scal4": np.ascontiguousarray(np.concatenate(
                [np.roll(taus, -r, axis=0), np.roll(R0dTaus, -r, axis=0)],
                axis=1)),
            # doubly-rolled Amat.T so PSUM row kk maps to global k=(kk+r)%N,
            # matching the rolled Ss rows; eye is roll-invariant.
            "statm2": np.ascontiguousarray(np.concatenate(
                [np.roll(np.roll(At, -r, axis=0), -r, axis=1), eye], axis=1)),
            "arow2": np.ascontiguousarray(np.concatenate(
                [Amat[r:r + S, :], eye[r:r + S, :]], axis=1)),
        })
    return maps


def _assemble(results):
    pred = np.empty((N, H, L), dtype=np.float32)
    signal = np.empty((N, H, L), dtype=np.float32)
    amatT = np.empty((N, N), dtype=np.float32)
    for c, res in enumerate(results):
        r = S * c
        for h in range(H):
            # pred_o rows are in rolled-k order; roll back by +r
            pred[:, h, LC * c:LC * (c + 1)] = np.roll(res["pred_o"][h], r, axis=0)
            signal[r:r + S, h, :] = res["sig_o"][h]
        amatT[r:r + S, :] = res["amt_o"]
    return pred, signal, amatT


def kernel(x, Amat, taus, R0dTaus):
    from concourse import bass_utils

    if "nc" not in _CACHE:
        _CACHE["nc"] = _build()
    res = bass_utils.run_bass_kernel_spmd(
        _CACHE["nc"], _in_maps(x, Amat, taus, R0dTaus), core_ids=list(range(NC))
    )
    return _assemble(res.results)


# revision 29
# speedup vs baseline: 2.2658x; 1.0786x over previous
"""Trainium2 Bass kernel for nn_EpisA (sparse_attention, 8 NeuronCores).

Math (reference):
    signal = relu(x)                                   (n,h,l)
    Ss     = 1 - cumsum(signal, -1)                    (n,h,l)
    Is[i]  = sum_{j<=i} r^(i-j) * signal[j],  r = 1 - 1/tau   -> linear scan
    alpha  = 1 - exp(-R0dTaus * Is)
    Alpha[k] = sum_m (Amat[k,m] + I[k,m]) * alpha[m]   (node mixing matmul)
    predSignal = Alpha * Ss
    outputs: (predSignal, signal, Amat + I)

Device strategy (identical SPMD program on 8 cores, no collectives — the
problem is tiny (~3 MB I/O) so a collective's latency floor would dominate):
  * every core computes the row-wise pipeline (relu / Is-scan / exp / Ss-scan)
    for all 256 (n,h) rows: scans run on the Vector engine via
    tensor_tensor_scan, transcendentals on the Scalar engine, relu on GPSIMD.
  * the node-mixing matmul is sharded over L: each core contracts only its own
    128-column chunk of e = exp(-R0d*Is), selected with a partition_id()-based
    dynamic slice.  Alpha = stat @ 1 - stat @ e via two accumulating matmuls.
  * signal / tempAmat.T outputs are sharded over node rows: inputs are rolled
    per core so each core's shard sits at partitions [0:16] of its tiles.
Host side only reorders/shards inputs and reassembles the three outputs.
"""

import numpy as np

for _p in ("/opt/trn_rl_repo", "/root/.axon_site/_ro/trn_rl_repo"):
    try:
        import concourse  # noqa: F401
        break
    except ImportError:
        import sys
        if _p not in sys.path:
            sys.path.insert(0, _p)

N, H, L = 128, 2, 1024
NC = 8            # cores
S = N // NC       # 16 node rows per core (signal / tempAmat.T shards)
LC = L // NC      # 128 L columns per core (predSignal shard)

_CACHE = {}


def _build():
    """Build + compile the (single, SPMD-identical) Bass program."""
    from concourse import bacc, bass, mybir, tile

    f32 = mybir.dt.float32
    Alu = mybir.AluOpType
    Act = mybir.ActivationFunctionType

    nc = bacc.Bacc(
        "TRN2", target_bir_lowering=False, debug=False, num_devices=NC
    )

    xh = nc.dram_tensor("xh", [H, N, L], f32, kind="ExternalInput").ap()
    # merged scalars: [taus | R0dTaus] and [rolled Amat.T | eye], [Amat rows | I rows]
    scal4 = nc.dram_tensor("scal4", [N, 2 * H], f32, kind="ExternalInput").ap()
    statm2 = nc.dram_tensor("statm2", [N, 2 * N], f32, kind="ExternalInput").ap()
    arow2 = nc.dram_tensor("arow2", [S, 2 * N], f32, kind="ExternalInput").ap()

    pred_o = nc.dram_tensor("pred_o", [H, N, LC], f32, kind="ExternalOutput").ap()
    sig_o = nc.dram_tensor("sig_o", [H, S, L], f32, kind="ExternalOutput").ap()
    amt_o = nc.dram_tensor("amt_o", [S, N], f32, kind="ExternalOutput").ap()

    import bass_rust

    def bcast(col_ap, n):
        """(P,1) column AP -> (P,n) stride-0 broadcast AP along free dim."""
        newap = [list(p) for p in col_ap.ap]
        newap[-1] = [0, n]
        return bass_rust.AP(col_ap.tensor, col_ap.offset, newap)

    with tile.TileContext(nc) as tc:
        with (
            tc.tile_pool(name="p", bufs=1) as pool,
            tc.tile_pool(name="ps", bufs=2, space="PSUM") as psp,
        ):
            def t(name, shape):
                return pool.tile(shape, f32, name=name, tag=name)

            # x first on the Sync sequencer: it gates the whole pipeline
            # (relu -> scans). Split into partition halves so each transfer
            # lands sooner; const loads go on the GPSIMD sequencer so their
            # ~600ns issue costs don't delay x. Scalar does only relu/exp;
            # engines run their streams in order, so nothing may precede the
            # relus in the Scalar stream.
            # h0 is loaded in two column-halves: the first 256KB gates the
            # whole scan pipeline, so get it on-chip ~2us sooner
            Lh = L // 2
            x_ts = []
            for h in range(H):
                x_t = t(f"x{h}", [N, L])
                if h == 0:
                    nc.sync.dma_start(x_t[:, 0:Lh], xh[h, :, 0:Lh])
                    nc.sync.dma_start(x_t[:, Lh:L], xh[h, :, Lh:L])
                else:
                    nc.sync.dma_start(x_t, xh[h])
                x_ts.append(x_t)

            scal_t = t("scal", [N, 2 * H]); nc.gpsimd.dma_start(scal_t, scal4)
            arow2_t = t("arow2", [S, 2 * N]); nc.gpsimd.dma_start(arow2_t, arow2)
            statm2_t = t("statm2", [N, 2 * N]); nc.gpsimd.dma_start(statm2_t, statm2)
            taus_t = scal_t[:, 0:H]
            r0d_t = scal_t[:, H:2 * H]
            statm_t = statm2_t[:, 0:N]
            eyem_t = statm2_t[:, N:2 * N]
            arow_t = arow2_t[:, 0:N]
            erow_t = arow2_t[:, N:2 * N]

            # r = 1 - 1/tau per (rolled) row, column h; -R0dTaus likewise.
            # (tiny DVE ops; must clear DVE before the scan block arrives)
            inv_t = t("inv", [N, H])
            nc.vector.reciprocal(inv_t, taus_t)
            rcol = t("rcol", [N, H])
            nc.vector.tensor_scalar(rcol, inv_t, -1.0, 1.0, Alu.mult, Alu.add)
            nr0d = t("nr0d", [N, H])
            nc.vector.tensor_scalar(nr0d, r0d_t, -1.0, None, Alu.mult)

            onec = t("onec", [N, 1])
            nc.gpsimd.memset(onec, 1.0)          # scan data0 broadcast source

            # dummy activation: pulls the ~1.3us ACT table load off the
            # critical path (it otherwise runs right before the first relu,
            # after the x-DMA wait)
            warm = t("warm", [1, 1])
            nc.vector.memset(warm, 0.0)
            warm2 = t("warm2", [1, 1])
            nc.scalar.activation(warm2, warm, Act.Exp)

            # dynamic L-chunk offset = 128 * core_id (registers on all engines)
            off = nc.partition_id() * LC



            # tempAmat.T output rows = Amat[shard,:] + I[shard,:] (GPSIMD:
            # runs early in parallel, keeps the DVE stream clear for scans)
            amrow = t("amrow", [S, N])
            nc.gpsimd.tensor_tensor(amrow, arow_t, erow_t, Alu.add)

            # column sums C[k] = sum_m statm[m,k] + 1, entirely on PE (the
            # eye matmul adds the +1); consumed straight from PSUM
            psc = psp.tile([N, 1], f32, name="psc", tag="psc")
            nc.tensor.matmul(psc, lhsT=statm_t, rhs=onec, start=True, stop=False)
            nc.tensor.matmul(psc, lhsT=eyem_t, rhs=onec, start=False, stop=True)

            # the critical chain: relu (Scalar) -> Is scan -> -Ss scan (DVE)
            # per h; chunk-exp (Scalar) + matmul (PE) overlap the later scans.
            #   Alpha = (Amat.T+I) @ (1-e) = C - (Amat.T+I) @ e
            #   pred  = Alpha * Ss = ((Amat.T+I)@e - C) * (-Ss)
            sig_ts, is_ts, ss_ts, ps_ts, pred_ts = [], [], [], [], []
            for h in range(H):
                sig_t = t(f"sig{h}", [N, L])
                if h == 0:
                    # relu + scans in chained pieces: start on the first
                    # x-quarter while the rest is still in flight
                    for lo, hi in ((0, Lh), (Lh, L)):
                        nc.scalar.activation(sig_t[:, lo:hi],
                                             x_ts[h][:, lo:hi], Act.Relu)
                else:
                    nc.scalar.activation(sig_t, x_ts[h], Act.Relu)
                sig_ts.append(sig_t)

                # Is[i] = r*Is[i-1] + sig[i]   (r broadcast via stride-0 AP)
                is_t = t(f"is{h}", [N, L])
                if h == 0:
                    prev = 0.0
                    for lo, hi in ((0, Lh), (Lh, L)):
                        nc.vector.tensor_tensor_scan(
                            is_t[:, lo:hi], bcast(rcol[:, h:h + 1], hi - lo),
                            sig_t[:, lo:hi], prev, Alu.mult, Alu.add
                        )
                        prev = is_t[:, hi - 1:hi]
                else:
                    nc.vector.tensor_tensor_scan(
                        is_t, bcast(rcol[:, h:h + 1], L), sig_t, 0.0,
                        Alu.mult, Alu.add
                    )
                is_ts.append(is_t)

                # -Ss[i] = -Ss[i-1] + sig[i], -Ss[-1] = -1  (= cumsum - 1)
                ss_t = t(f"ssn{h}", [N, L])
                if h == 0:
                    prev = -1.0
                    for lo, hi in ((0, 512), (512, L)):
                        nc.vector.tensor_tensor_scan(
                            ss_t[:, lo:hi], bcast(onec[:, 0:1], hi - lo),
                            sig_t[:, lo:hi], prev, Alu.mult, Alu.add
                        )
                        prev = ss_t[:, hi - 1:hi]
                else:
                    nc.vector.tensor_tensor_scan(
                        ss_t, bcast(onec[:, 0:1], L), sig_t, -1.0,
                        Alu.mult, Alu.add
                    )
                ss_ts.append(ss_t)

                # e = exp(-R0d*Is) on own L-chunk ONLY (nothing else reads e)
                e_t = t(f"e{h}", [N, LC])
                nc.scalar.activation(e_t, is_t[:, bass.ds(off, LC)], Act.Exp,
                                     scale=nr0d[:, h:h + 1])

                # (Amat.T + I) @ e via two PSUM-accumulating matmuls
                ps = psp.tile([N, LC], f32, name=f"ps{h}", tag=f"ps{h}")
                nc.tensor.matmul(ps, lhsT=statm_t, rhs=e_t,
                                 start=True, stop=False)
                nc.tensor.matmul(ps, lhsT=eyem_t, rhs=e_t,
                                 start=False, stop=True)
                ps_ts.append(ps)

            # preds AFTER the whole scan block in the DVE stream (a PSUM wait
            # must never stall a pending scan): pred = (psE - C) * (-Ss)
            for h in range(H):
                pred_t = t(f"pred{h}", [N, LC])
                nc.vector.scalar_tensor_tensor(
                    pred_t, ps_ts[h], psc[:, 0:1],
                    ss_ts[h][:, bass.ds(off, LC)],
                    Alu.subtract, Alu.mult
                )
                pred_ts.append(pred_t)

            # output DMAs on the (in-order) Sync stream: earliest-ready first
            for h in range(H):
                nc.sync.dma_start(sig_o[h], sig_ts[h][0:S, :])
            for h in range(H):
                nc.sync.dma_start(pred_o[h], pred_ts[h])
            nc.sync.dma_start(amt_o, amrow)

    nc.compile()
    return nc


def _in_maps(x, Amat, taus, R0dTaus):
    """Per-core host-side sharding: pure slicing/rolling, no math."""
    x = np.ascontiguousarray(x, dtype=np.float32)
    Amat = np.ascontiguousarray(Amat, dtype=np.float32)
    taus = np.ascontiguousarray(taus, dtype=np.float32)
    R0dTaus = np.ascontiguousarray(R0dTaus, dtype=np.float32)
    xt = x.transpose(1, 0, 2)          # (H, N, L)
    At = Amat.T.copy()                 # At[m, k] = Amat[k, m]
    eye = np.eye(N, dtype=np.float32)
    maps = []
    for c in range(NC):
        r = S * c
        maps.append({
            # row-rolled so this core's node shard sits at partitions [0:S)
            "xh": np.ascontiguousarray(np.roll(xt, -r, axis=1)),
            "scal4": np.ascontiguousarray(np.concatenate(
                [np.roll(taus, -r, axis=0), np.roll(R0dTaus, -r, axis=0)],
                axis=1)),
            # doubly-rolled Amat.T so PSUM row kk maps to global k=(kk+r)%N,
            # matching the rolled Ss rows; eye is roll-invariant.
            "statm2": np.ascontiguousarray(np.concatenate(
                [np.roll(np.roll(At, -r, axis=0), -r, axis=1), eye], axis=1)),
            "arow2": np.ascontiguousarray(np.concatenate(
                [Amat[r:r + S, :], eye[r:r + S, :]], axis=1)),
        })
    return maps


def _assemble(results):
    pred = np.empty((N, H, L), dtype=np.float32)
    signal = np.empty((N, H, L), dtype=np.float32)
    amatT = np.empty((N, N), dtype=np.float32)
    for c, res in enumerate(results):
        r = S * c
        for h in range(H):
            # pred_o rows are in rolled-k order; roll back by +r
            pred[:, h, LC * c:LC * (c + 1)] = np.roll(res["pred_o"][h], r, axis=0)
            signal[r:r + S, h, :] = res["sig_o"][h]
        amatT[r:r + S, :] = res["amt_o"]
    return pred, signal, amatT


def kernel(x, Amat, taus, R0dTaus):
    from concourse import bass_utils

    if "nc" not in _CACHE:
        _CACHE["nc"] = _build()
    res = bass_utils.run_bass_kernel_spmd(
        _CACHE["nc"], _in_maps(x, Amat, taus, R0dTaus), core_ids=list(range(NC))
    )
    return _assemble(res.results)


# revision 30
# speedup vs baseline: 2.2676x; 1.0008x over previous
"""Trainium2 Bass kernel for nn_EpisA (sparse_attention, 8 NeuronCores).

Math (reference):
    signal = relu(x)                                   (n,h,l)
    Ss     = 1 - cumsum(signal, -1)                    (n,h,l)
    Is[i]  = sum_{j<=i} r^(i-j) * signal[j],  r = 1 - 1/tau   -> linear scan
    alpha  = 1 - exp(-R0dTaus * Is)
    Alpha[k] = sum_m (Amat[k,m] + I[k,m]) * alpha[m]   (node mixing matmul)
    predSignal = Alpha * Ss
    outputs: (predSignal, signal, Amat + I)

Device strategy (identical SPMD program on 8 cores, no collectives — the
problem is tiny (~3 MB I/O) so a collective's latency floor would dominate):
  * every core computes the row-wise pipeline (relu / Is-scan / exp / Ss-scan)
    for all 256 (n,h) rows: scans run on the Vector engine via
    tensor_tensor_scan, transcendentals on the Scalar engine, relu on GPSIMD.
  * the node-mixing matmul is sharded over L: each core contracts only its own
    128-column chunk of e = exp(-R0d*Is), selected with a partition_id()-based
    dynamic slice.  Alpha = stat @ 1 - stat @ e via two accumulating matmuls.
  * signal / tempAmat.T outputs are sharded over node rows: inputs are rolled
    per core so each core's shard sits at partitions [0:16] of its tiles.
Host side only reorders/shards inputs and reassembles the three outputs.
"""

import numpy as np

for _p in ("/opt/trn_rl_repo", "/root/.axon_site/_ro/trn_rl_repo"):
    try:
        import concourse  # noqa: F401
        break
    except ImportError:
        import sys
        if _p not in sys.path:
            sys.path.insert(0, _p)

N, H, L = 128, 2, 1024
NC = 8            # cores
S = N // NC       # 16 node rows per core (signal / tempAmat.T shards)
LC = L // NC      # 128 L columns per core (predSignal shard)

_CACHE = {}


def _build():
    """Build + compile the (single, SPMD-identical) Bass program."""
    from concourse import bacc, bass, mybir, tile

    f32 = mybir.dt.float32
    Alu = mybir.AluOpType
    Act = mybir.ActivationFunctionType

    nc = bacc.Bacc(
        "TRN2", target_bir_lowering=False, debug=False, num_devices=NC
    )

    xh = nc.dram_tensor("xh", [H, N, L], f32, kind="ExternalInput").ap()
    # merged scalars: [taus | R0dTaus] and [rolled Amat.T | eye], [Amat rows | I rows]
    scal4 = nc.dram_tensor("scal4", [N, 2 * H], f32, kind="ExternalInput").ap()
    statm2 = nc.dram_tensor("statm2", [N, 2 * N], f32, kind="ExternalInput").ap()
    arow2 = nc.dram_tensor("arow2", [S, 2 * N], f32, kind="ExternalInput").ap()

    pred_o = nc.dram_tensor("pred_o", [H, N, LC], f32, kind="ExternalOutput").ap()
    sig_o = nc.dram_tensor("sig_o", [H, S, L], f32, kind="ExternalOutput").ap()
    amt_o = nc.dram_tensor("amt_o", [S, N], f32, kind="ExternalOutput").ap()

    import bass_rust

    def bcast(col_ap, n):
        """(P,1) column AP -> (P,n) stride-0 broadcast AP along free dim."""
        newap = [list(p) for p in col_ap.ap]
        newap[-1] = [0, n]
        return bass_rust.AP(col_ap.tensor, col_ap.offset, newap)

    with tile.TileContext(nc) as tc:
        with (
            tc.tile_pool(name="p", bufs=1) as pool,
            tc.tile_pool(name="ps", bufs=2, space="PSUM") as psp,
        ):
            def t(name, shape):
                return pool.tile(shape, f32, name=name, tag=name)

            # x first on the Sync sequencer: it gates the whole pipeline
            # (relu -> scans). Split into partition halves so each transfer
            # lands sooner; const loads go on the GPSIMD sequencer so their
            # ~600ns issue costs don't delay x. Scalar does only relu/exp;
            # engines run their streams in order, so nothing may precede the
            # relus in the Scalar stream.
            # h0 is loaded in two column-halves: the first 256KB gates the
            # whole scan pipeline, so get it on-chip ~2us sooner
            Lh = L // 2
            x_ts = []
            for h in range(H):
                x_t = t(f"x{h}", [N, L])
                if h == 0:
                    nc.sync.dma_start(x_t[:, 0:Lh], xh[h, :, 0:Lh])
                    nc.sync.dma_start(x_t[:, Lh:L], xh[h, :, Lh:L])
                else:
                    nc.sync.dma_start(x_t, xh[h])
                x_ts.append(x_t)

            scal_t = t("scal", [N, 2 * H]); nc.gpsimd.dma_start(scal_t, scal4)
            arow2_t = t("arow2", [S, 2 * N]); nc.gpsimd.dma_start(arow2_t, arow2)
            statm2_t = t("statm2", [N, 2 * N]); nc.gpsimd.dma_start(statm2_t, statm2)
            taus_t = scal_t[:, 0:H]
            r0d_t = scal_t[:, H:2 * H]
            statm_t = statm2_t[:, 0:N]
            eyem_t = statm2_t[:, N:2 * N]
            arow_t = arow2_t[:, 0:N]
            erow_t = arow2_t[:, N:2 * N]

            # r = 1 - 1/tau per (rolled) row, column h; -R0dTaus likewise.
            # (tiny DVE ops; must clear DVE before the scan block arrives)
            inv_t = t("inv", [N, H])
            nc.vector.reciprocal(inv_t, taus_t)
            rcol = t("rcol", [N, H])
            nc.vector.tensor_scalar(rcol, inv_t, -1.0, 1.0, Alu.mult, Alu.add)
            nr0d = t("nr0d", [N, H])
            nc.vector.tensor_scalar(nr0d, r0d_t, -1.0, None, Alu.mult)

            onec = t("onec", [N, 1])
            nc.gpsimd.memset(onec, 1.0)          # scan data0 broadcast source

            # dummy activation: pulls the ~1.3us ACT table load off the
            # critical path (it otherwise runs right before the first relu,
            # after the x-DMA wait)
            warm = t("warm", [1, 1])
            nc.vector.memset(warm, 0.0)
            warm2 = t("warm2", [1, 1])
            nc.scalar.activation(warm2, warm, Act.Exp)

            # dynamic L-chunk offset = 128 * core_id (registers on all engines)
            off = nc.partition_id() * LC




            # column sums C[k] = sum_m statm[m,k] + 1, entirely on PE (the
            # eye matmul adds the +1); consumed straight from PSUM
            psc = psp.tile([N, 1], f32, name="psc", tag="psc")
            nc.tensor.matmul(psc, lhsT=statm_t, rhs=onec, start=True, stop=False)
            nc.tensor.matmul(psc, lhsT=eyem_t, rhs=onec, start=False, stop=True)

            # the critical chain: relu (Scalar) -> Is scan -> -Ss scan (DVE)
            # per h; chunk-exp (Scalar) + matmul (PE) overlap the later scans.
            #   Alpha = (Amat.T+I) @ (1-e) = C - (Amat.T+I) @ e
            #   pred  = Alpha * Ss = ((Amat.T+I)@e - C) * (-Ss)
            sig_ts, is_ts, ss_ts, ps_ts, pred_ts = [], [], [], [], []
            for h in range(H):
                sig_t = t(f"sig{h}", [N, L])
                if h == 0:
                    # relu + scans in chained pieces: start on the first
                    # x-quarter while the rest is still in flight
                    for lo, hi in ((0, Lh), (Lh, L)):
                        nc.scalar.activation(sig_t[:, lo:hi],
                                             x_ts[h][:, lo:hi], Act.Relu)
                else:
                    nc.scalar.activation(sig_t, x_ts[h], Act.Relu)
                sig_ts.append(sig_t)

                # Is[i] = r*Is[i-1] + sig[i]   (r broadcast via stride-0 AP)
                is_t = t(f"is{h}", [N, L])
                if h == 0:
                    prev = 0.0
                    for lo, hi in ((0, Lh), (Lh, L)):
                        nc.vector.tensor_tensor_scan(
                            is_t[:, lo:hi], bcast(rcol[:, h:h + 1], hi - lo),
                            sig_t[:, lo:hi], prev, Alu.mult, Alu.add
                        )
                        prev = is_t[:, hi - 1:hi]
                else:
                    nc.vector.tensor_tensor_scan(
                        is_t, bcast(rcol[:, h:h + 1], L), sig_t, 0.0,
                        Alu.mult, Alu.add
                    )
                is_ts.append(is_t)

                # -Ss[i] = -Ss[i-1] + sig[i], -Ss[-1] = -1  (= cumsum - 1)
                ss_t = t(f"ssn{h}", [N, L])
                if h == 0:
                    prev = -1.0
                    for lo, hi in ((0, 512), (512, L)):
                        nc.vector.tensor_tensor_scan(
                            ss_t[:, lo:hi], bcast(onec[:, 0:1], hi - lo),
                            sig_t[:, lo:hi], prev, Alu.mult, Alu.add
                        )
                        prev = ss_t[:, hi - 1:hi]
                else:
                    last_scan = nc.vector.tensor_tensor_scan(
                        ss_t, bcast(onec[:, 0:1], L), sig_t, -1.0,
                        Alu.mult, Alu.add
                    )
                ss_ts.append(ss_t)

                # e = exp(-R0d*Is) on own L-chunk ONLY (nothing else reads e)
                e_t = t(f"e{h}", [N, LC])
                nc.scalar.activation(e_t, is_t[:, bass.ds(off, LC)], Act.Exp,
                                     scale=nr0d[:, h:h + 1])

                # (Amat.T + I) @ e via two PSUM-accumulating matmuls
                ps = psp.tile([N, LC], f32, name=f"ps{h}", tag=f"ps{h}")
                nc.tensor.matmul(ps, lhsT=statm_t, rhs=e_t,
                                 start=True, stop=False)
                nc.tensor.matmul(ps, lhsT=eyem_t, rhs=e_t,
                                 start=False, stop=True)
                ps_ts.append(ps)

            # preds AFTER the whole scan block in the DVE stream (a PSUM wait
            # must never stall a pending scan): pred = (psE - C) * (-Ss)
            for h in range(H):
                pred_t = t(f"pred{h}", [N, LC])
                nc.vector.scalar_tensor_tensor(
                    pred_t, ps_ts[h], psc[:, 0:1],
                    ss_ts[h][:, bass.ds(off, LC)],
                    Alu.subtract, Alu.mult
                )
                pred_ts.append(pred_t)

            # tempAmat.T output rows = Amat[shard,:] + I[shard,:] -- on DVE,
            # explicitly ordered after the last scan so the static scheduler
            # can never hoist it (and its input-DMA wait) above the scan block
            amrow = t("amrow", [S, N])
            am_inst = nc.vector.tensor_tensor(amrow, arow_t, erow_t, Alu.add)
            from concourse.tile import add_dep_helper
            add_dep_helper(am_inst.ins, last_scan.ins, sync=False,
                           reason="amrow after scan block")

            # output DMAs on the (in-order) Sync stream: earliest-ready first
            for h in range(H):
                nc.sync.dma_start(sig_o[h], sig_ts[h][0:S, :])
            for h in range(H):
                nc.sync.dma_start(pred_o[h], pred_ts[h])
            nc.sync.dma_start(amt_o, amrow)

    nc.compile()
    return nc


def _in_maps(x, Amat, taus, R0dTaus):
    """Per-core host-side sharding: pure slicing/rolling, no math."""
    x = np.ascontiguousarray(x, dtype=np.float32)
    Amat = np.ascontiguousarray(Amat, dtype=np.float32)
    taus = np.ascontiguousarray(taus, dtype=np.float32)
    R0dTaus = np.ascontiguousarray(R0dTaus, dtype=np.float32)
    xt = x.transpose(1, 0, 2)          # (H, N, L)
    At = Amat.T.copy()                 # At[m, k] = Amat[k, m]
    eye = np.eye(N, dtype=np.float32)
    maps = []
    for c in range(NC):
        r = S * c
        maps.append({
            # row-rolled so this core's node shard sits at partitions [0:S)
            "xh": np.ascontiguousarray(np.roll(xt, -r, axis=1)),
            "scal4": np.ascontiguousarray(np.concatenate(
                [np.roll(taus, -r, axis=0), np.roll(R0dTaus, -r, axis=0)],
                axis=1)),
            # doubly-rolled Amat.T so PSUM row kk maps to global k=(kk+r)%N,
            # matching the rolled Ss rows; eye is roll-invariant.
            "statm2": np.ascontiguousarray(np.concatenate(
                [np.roll(np.roll(At, -r, axis=0), -r, axis=1), eye], axis=1)),
            "arow2": np.ascontiguousarray(np.concatenate(
                [Amat[r:r + S, :], eye[r:r + S, :]], axis=1)),
        })
    return maps


def _assemble(results):
    pred = np.empty((N, H, L), dtype=np.float32)
    signal = np.empty((N, H, L), dtype=np.float32)
    amatT = np.empty((N, N), dtype=np.float32)
    for c, res in enumerate(results):
        r = S * c
        for h in range(H):
            # pred_o rows are in rolled-k order; roll back by +r
            pred[:, h, LC * c:LC * (c + 1)] = np.roll(res["pred_o"][h], r, axis=0)
            signal[r:r + S, h, :] = res["sig_o"][h]
        amatT[r:r + S, :] = res["amt_o"]
    return pred, signal, amatT


def kernel(x, Amat, taus, R0dTaus):
    from concourse import bass_utils

    if "nc" not in _CACHE:
        _CACHE["nc"] = _build()
    res = bass_utils.run_bass_kernel_spmd(
        _CACHE["nc"], _in_maps(x, Amat, taus, R0dTaus), core_ids=list(range(NC))
    )
    return _assemble(res.results)


# revision 31
# speedup vs baseline: 2.2742x; 1.0029x over previous
"""Trainium2 Bass kernel for nn_EpisA (sparse_attention, 8 NeuronCores).

Math (reference):
    signal = relu(x)                                   (n,h,l)
    Ss     = 1 - cumsum(signal, -1)                    (n,h,l)
    Is[i]  = sum_{j<=i} r^(i-j) * signal[j],  r = 1 - 1/tau   -> linear scan
    alpha  = 1 - exp(-R0dTaus * Is)
    Alpha[k] = sum_m (Amat[k,m] + I[k,m]) * alpha[m]   (node mixing matmul)
    predSignal = Alpha * Ss
    outputs: (predSignal, signal, Amat + I)

Device strategy (identical SPMD program on 8 cores, no collectives — the
problem is tiny (~3 MB I/O) so a collective's latency floor would dominate):
  * every core computes the row-wise pipeline (relu / Is-scan / exp / Ss-scan)
    for all 256 (n,h) rows: scans run on the Vector engine via
    tensor_tensor_scan, transcendentals on the Scalar engine, relu on GPSIMD.
  * the node-mixing matmul is sharded over L: each core contracts only its own
    128-column chunk of e = exp(-R0d*Is), selected with a partition_id()-based
    dynamic slice.  Alpha = stat @ 1 - stat @ e via two accumulating matmuls.
  * signal / tempAmat.T outputs are sharded over node rows: inputs are rolled
    per core so each core's shard sits at partitions [0:16] of its tiles.
Host side only reorders/shards inputs and reassembles the three outputs.
"""

import numpy as np

for _p in ("/opt/trn_rl_repo", "/root/.axon_site/_ro/trn_rl_repo"):
    try:
        import concourse  # noqa: F401
        break
    except ImportError:
        import sys
        if _p not in sys.path:
            sys.path.insert(0, _p)

N, H, L = 128, 2, 1024
NC = 8            # cores
S = N // NC       # 16 node rows per core (signal / tempAmat.T shards)
LC = L // NC      # 128 L columns per core (predSignal shard)

_CACHE = {}


def _build():
    """Build + compile the (single, SPMD-identical) Bass program."""
    from concourse import bacc, bass, mybir, tile

    f32 = mybir.dt.float32
    Alu = mybir.AluOpType
    Act = mybir.ActivationFunctionType

    nc = bacc.Bacc(
        "TRN2", target_bir_lowering=False, debug=False, num_devices=NC
    )

    xh = nc.dram_tensor("xh", [H, N, L], f32, kind="ExternalInput").ap()
    # merged scalars: [taus | R0dTaus] and [rolled Amat.T | eye], [Amat rows | I rows]
    scal4 = nc.dram_tensor("scal4", [N, 2 * H], f32, kind="ExternalInput").ap()
    statm = nc.dram_tensor("statm", [N, N], f32, kind="ExternalInput").ap()
    arow2 = nc.dram_tensor("arow2", [S, 2 * N], f32, kind="ExternalInput").ap()

    pred_o = nc.dram_tensor("pred_o", [H, N, LC], f32, kind="ExternalOutput").ap()
    sig_o = nc.dram_tensor("sig_o", [H, S, L], f32, kind="ExternalOutput").ap()
    amt_o = nc.dram_tensor("amt_o", [S, N], f32, kind="ExternalOutput").ap()

    import bass_rust

    def bcast(col_ap, n):
        """(P,1) column AP -> (P,n) stride-0 broadcast AP along free dim."""
        newap = [list(p) for p in col_ap.ap]
        newap[-1] = [0, n]
        return bass_rust.AP(col_ap.tensor, col_ap.offset, newap)

    with tile.TileContext(nc) as tc:
        with (
            tc.tile_pool(name="p", bufs=1) as pool,
            tc.tile_pool(name="ps", bufs=2, space="PSUM") as psp,
        ):
            def t(name, shape):
                return pool.tile(shape, f32, name=name, tag=name)

            # x first on the Sync sequencer: it gates the whole pipeline
            # (relu -> scans). Split into partition halves so each transfer
            # lands sooner; const loads go on the GPSIMD sequencer so their
            # ~600ns issue costs don't delay x. Scalar does only relu/exp;
            # engines run their streams in order, so nothing may precede the
            # relus in the Scalar stream.
            # h0 is loaded in two column-halves: the first 256KB gates the
            # whole scan pipeline, so get it on-chip ~2us sooner
            Lh = L // 2
            x_ts = []
            for h in range(H):
                x_t = t(f"x{h}", [N, L])
                if h == 0:
                    nc.sync.dma_start(x_t[:, 0:Lh], xh[h, :, 0:Lh])
                    nc.sync.dma_start(x_t[:, Lh:L], xh[h, :, Lh:L])
                else:
                    nc.sync.dma_start(x_t, xh[h])
                x_ts.append(x_t)

            scal_t = t("scal", [N, 2 * H]); nc.gpsimd.dma_start(scal_t, scal4)
            arow2_t = t("arow2", [S, 2 * N]); nc.gpsimd.dma_start(arow2_t, arow2)
            statm_t = t("statm", [N, N]); nc.gpsimd.dma_start(statm_t, statm)
            taus_t = scal_t[:, 0:H]
            r0d_t = scal_t[:, H:2 * H]
            arow_t = arow2_t[:, 0:N]
            erow_t = arow2_t[:, N:2 * N]

            # r = 1 - 1/tau per (rolled) row, column h; -R0dTaus likewise.
            # (tiny DVE ops; must clear DVE before the scan block arrives)
            inv_t = t("inv", [N, H])
            nc.vector.reciprocal(inv_t, taus_t)
            rcol = t("rcol", [N, H])
            nc.vector.tensor_scalar(rcol, inv_t, -1.0, 1.0, Alu.mult, Alu.add)
            nr0d = t("nr0d", [N, H])
            nc.vector.tensor_scalar(nr0d, r0d_t, -1.0, None, Alu.mult)

            onec = t("onec", [N, 1])
            nc.gpsimd.memset(onec, 1.0)          # scan data0 broadcast source

            # 128x128 identity built on-device (saves 64KB of input DMA):
            # keep broadcast-1 where (j - p) == 0, else 0
            eyem_t = t("eyem", [N, N])
            nc.gpsimd.affine_select(
                eyem_t, bcast(onec[:, 0:1], N), [[1, N]],
                mybir.AluOpType.is_equal, 0.0, base=0, channel_multiplier=-1
            )

            # dummy activation: pulls the ~1.3us ACT table load off the
            # critical path (it otherwise runs right before the first relu,
            # after the x-DMA wait)
            warm = t("warm", [1, 1])
            nc.vector.memset(warm, 0.0)
            warm2 = t("warm2", [1, 1])
            nc.scalar.activation(warm2, warm, Act.Exp)

            # dynamic L-chunk offset = 128 * core_id (registers on all engines)
            off = nc.partition_id() * LC




            # column sums C[k] = sum_m statm[m,k] + 1, entirely on PE (the
            # eye matmul adds the +1); consumed straight from PSUM
            psc = psp.tile([N, 1], f32, name="psc", tag="psc")
            nc.tensor.matmul(psc, lhsT=statm_t, rhs=onec, start=True, stop=False)
            nc.tensor.matmul(psc, lhsT=eyem_t, rhs=onec, start=False, stop=True)

            # the critical chain: relu (Scalar) -> Is scan -> -Ss scan (DVE)
            # per h; chunk-exp (Scalar) + matmul (PE) overlap the later scans.
            #   Alpha = (Amat.T+I) @ (1-e) = C - (Amat.T+I) @ e
            #   pred  = Alpha * Ss = ((Amat.T+I)@e - C) * (-Ss)
            sig_ts, is_ts, ss_ts, ps_ts, pred_ts = [], [], [], [], []
            for h in range(H):
                sig_t = t(f"sig{h}", [N, L])
                if h == 0:
                    # relu + scans in chained pieces: start on the first
                    # x-quarter while the rest is still in flight
                    for lo, hi in ((0, Lh), (Lh, L)):
                        nc.scalar.activation(sig_t[:, lo:hi],
                                             x_ts[h][:, lo:hi], Act.Relu)
                else:
                    nc.scalar.activation(sig_t, x_ts[h], Act.Relu)
                sig_ts.append(sig_t)

                # Is[i] = r*Is[i-1] + sig[i]   (r broadcast via stride-0 AP)
                is_t = t(f"is{h}", [N, L])
                if h == 0:
                    prev = 0.0
                    for lo, hi in ((0, Lh), (Lh, L)):
                        nc.vector.tensor_tensor_scan(
                            is_t[:, lo:hi], bcast(rcol[:, h:h + 1], hi - lo),
                            sig_t[:, lo:hi], prev, Alu.mult, Alu.add
                        )
                        prev = is_t[:, hi - 1:hi]
                else:
                    nc.vector.tensor_tensor_scan(
                        is_t, bcast(rcol[:, h:h + 1], L), sig_t, 0.0,
                        Alu.mult, Alu.add
                    )
                is_ts.append(is_t)

                # -Ss[i] = -Ss[i-1] + sig[i], -Ss[-1] = -1  (= cumsum - 1)
                ss_t = t(f"ssn{h}", [N, L])
                if h == 0:
                    prev = -1.0
                    for lo, hi in ((0, 512), (512, L)):
                        nc.vector.tensor_tensor_scan(
                            ss_t[:, lo:hi], bcast(onec[:, 0:1], hi - lo),
                            sig_t[:, lo:hi], prev, Alu.mult, Alu.add
                        )
                        prev = ss_t[:, hi - 1:hi]
                else:
                    last_scan = nc.vector.tensor_tensor_scan(
                        ss_t, bcast(onec[:, 0:1], L), sig_t, -1.0,
                        Alu.mult, Alu.add
                    )
                ss_ts.append(ss_t)

                # e = exp(-R0d*Is) on own L-chunk ONLY (nothing else reads e)
                e_t = t(f"e{h}", [N, LC])
                nc.scalar.activation(e_t, is_t[:, bass.ds(off, LC)], Act.Exp,
                                     scale=nr0d[:, h:h + 1])

                # (Amat.T + I) @ e via two PSUM-accumulating matmuls
                ps = psp.tile([N, LC], f32, name=f"ps{h}", tag=f"ps{h}")
                nc.tensor.matmul(ps, lhsT=statm_t, rhs=e_t,
                                 start=True, stop=False)
                nc.tensor.matmul(ps, lhsT=eyem_t, rhs=e_t,
                                 start=False, stop=True)
                ps_ts.append(ps)

            # preds AFTER the whole scan block in the DVE stream (a PSUM wait
            # must never stall a pending scan): pred = (psE - C) * (-Ss)
            for h in range(H):
                pred_t = t(f"pred{h}", [N, LC])
                nc.vector.scalar_tensor_tensor(
                    pred_t, ps_ts[h], psc[:, 0:1],
                    ss_ts[h][:, bass.ds(off, LC)],
                    Alu.subtract, Alu.mult
                )
                pred_ts.append(pred_t)

            # tempAmat.T output rows = Amat[shard,:] + I[shard,:] -- on DVE,
            # explicitly ordered after the last scan so the static scheduler
            # can never hoist it (and its input-DMA wait) above the scan block
            amrow = t("amrow", [S, N])
            am_inst = nc.vector.tensor_tensor(amrow, arow_t, erow_t, Alu.add)
            from concourse.tile import add_dep_helper
            add_dep_helper(am_inst.ins, last_scan.ins, sync=False,
                           reason="amrow after scan block")

            # output DMAs on the (in-order) Sync stream: earliest-ready first
            for h in range(H):
                nc.sync.dma_start(sig_o[h], sig_ts[h][0:S, :])
            for h in range(H):
                nc.sync.dma_start(pred_o[h], pred_ts[h])
            nc.sync.dma_start(amt_o, amrow)

    nc.compile()
    return nc


def _in_maps(x, Amat, taus, R0dTaus):
    """Per-core host-side sharding: pure slicing/rolling, no math."""
    x = np.ascontiguousarray(x, dtype=np.float32)
    Amat = np.ascontiguousarray(Amat, dtype=np.float32)
    taus = np.ascontiguousarray(taus, dtype=np.float32)
    R0dTaus = np.ascontiguousarray(R0dTaus, dtype=np.float32)
    xt = x.transpose(1, 0, 2)          # (H, N, L)
    At = Amat.T.copy()                 # At[m, k] = Amat[k, m]
    eye = np.eye(N, dtype=np.float32)
    maps = []
    for c in range(NC):
        r = S * c
        maps.append({
            # row-rolled so this core's node shard sits at partitions [0:S)
            "xh": np.ascontiguousarray(np.roll(xt, -r, axis=1)),
            "scal4": np.ascontiguousarray(np.concatenate(
                [np.roll(taus, -r, axis=0), np.roll(R0dTaus, -r, axis=0)],
                axis=1)),
            # doubly-rolled Amat.T so PSUM row kk maps to global k=(kk+r)%N,
            # matching the rolled Ss rows; eye is roll-invariant.
            "statm": np.ascontiguousarray(
                np.roll(np.roll(At, -r, axis=0), -r, axis=1)),
            "arow2": np.ascontiguousarray(np.concatenate(
                [Amat[r:r + S, :], eye[r:r + S, :]], axis=1)),
        })
    return maps


def _assemble(results):
    pred = np.empty((N, H, L), dtype=np.float32)
    signal = np.empty((N, H, L), dtype=np.float32)
    amatT = np.empty((N, N), dtype=np.float32)
    for c, res in enumerate(results):
        r = S * c
        for h in range(H):
            # pred_o rows are in rolled-k order; roll back by +r
            pred[:, h, LC * c:LC * (c + 1)] = np.roll(res["pred_o"][h], r, axis=0)
            signal[r:r + S, h, :] = res["sig_o"][h]
        amatT[r:r + S, :] = res["amt_o"]
    return pred, signal, amatT


def kernel(x, Amat, taus, R0dTaus):
    from concourse import bass_utils

    if "nc" not in _CACHE:
        _CACHE["nc"] = _build()
    res = bass_utils.run_bass_kernel_spmd(
        _CACHE["nc"], _in_maps(x, Amat, taus, R0dTaus), core_ids=list(range(NC))
    )
    return _assemble(res.results)
